# revision 58
# baseline (speedup 1.0000x reference)
"""PointFlow2DVAE loss kernel for 8 Trainium2 NeuronCores.

Data-parallel over batch B=8: one point cloud per core. Each core runs the
PointNet encoder, the combined Euler CNF integration (forward + generation
branch) with the exact-divergence computation folded into precomputed
matmuls, and the chamfer distance. Cores emit small partial-sum vectors;
the final scalar combine happens on host.

Perf structure:
- The two big K=256 matmuls per CNF eval (W2@h1, Wu@h1sq) and the velocity
  matmul (W3@h2) run as fp8e4m3 DoubleRow matmuls (0.5 PE cycles/row,
  K=256 folded into one call). h1/h2/h1sq are produced in fp8 directly.
- Chamfer distance uses bf16 operands with exact-product compensation:
  points are rounded to bf16 (a benign smooth shift of both clouds) and
  |r|^2/|x|^2 rows are hi/lo bf16 split so every PE product is exact in
  fp32 accumulation. Both chamfer directions are computed as separate
  matmul passes + free-axis min reductions (no PE transposes).
- Elementwise div-chain work is split across DVE (h1sq) and Pool
  (h2sq, scr) to balance engine load.
"""
import numpy as np
import ml_dtypes

import concourse.bacc as bacc
import concourse.bass as bass
import concourse.tile as tile
from concourse import mybir
from concourse.bass_utils import run_bass_kernel_spmd

B, N, D = 8, 2048, 2
LAT, ENC_H, CNF_H = 128, 256, 256
STEPS = 10
DT = 1.0 / STEPS
LAM_R, LAM_P, LAM_E, LAM_C, LAM_V = 1.0, 0.1, 0.01, 10.0, 0.01
LOG2PI = float(np.log(2.0 * np.pi))

NT = 512
NNT = N // NT
import os
PHASES = os.environ.get("KPHASES", "enc,cnf,cham").split(",")
F32 = mybir.dt.float32
MDT = mybir.dt.float32r
F8 = mybir.dt.float8e4
BF16 = mybir.dt.bfloat16
NP_F8 = mybir.dt.np(F8)
NP_BF16 = mybir.dt.np(BF16)

AF = mybir.ActivationFunctionType
ALU = mybir.AluOpType
AX = mybir.AxisListType
DR = mybir.MatmulPerfMode.DoubleRow


def _mm(ap):
    return ap


def host_precompute(w):
    f = np.float32
    W1, b1, W2, b2, W3, b3 = w["W1"], w["b1"], w["W2"], w["b2"], w["W3"], w["b3"]
    pre = {}
    pre["enc1"] = np.ascontiguousarray(
        np.concatenate([w["We1"].T, w["be1"][None, :]], 0), f)       # [3,256]
    pre["We2T"] = np.ascontiguousarray(
        w["We2"].T.reshape(2, 128, 256).transpose(1, 0, 2), f)        # [128,2,256]
    pre["be2r"] = np.ascontiguousarray(w["be2"][None, :], f)          # [1,256]
    pre["WmuT"] = np.ascontiguousarray(
        w["Wmu"].T.reshape(2, 128, 128).transpose(1, 0, 2), f)        # [128,2,128]
    pre["WlvT"] = np.ascontiguousarray(
        w["Wlv"].T.reshape(2, 128, 128).transpose(1, 0, 2), f)
    pre["bmulv"] = np.ascontiguousarray(
        np.stack([w["bmu"], w["blv"]], 1), f)                         # [128,2]

    W1p = W1[:, 0:2]
    W1t = W1[:, 2]
    pre["W1zT"] = np.ascontiguousarray(W1[:, 3:].T, f)                # [128,256]
    # a1top: W1p.T repeated for 20 evals x 2 m-blocks: [2, 20, 2, 128]
    pre["a1top"] = np.ascontiguousarray(
        np.broadcast_to(W1p.T.reshape(2, 1, 2, 128), (2, 20, 2, 128)), f)
    idx = np.arange(STEPS)
    pb3 = W1p @ b3
    TB_e = (idx * DT)[:, None] * W1t[None, :] + b1[None, :] \
        + (idx * DT)[:, None] * pb3[None, :]
    TB_g = (1.0 - idx * DT)[:, None] * W1t[None, :] + b1[None, :] \
        - (idx * DT)[:, None] * pb3[None, :]
    pre["TB"] = np.ascontiguousarray(np.concatenate([TB_e, TB_g], 0), f)  # [20,256]

    pre["W28"] = np.ascontiguousarray(
        W2.T.reshape(2, 128, 256).transpose(1, 0, 2)).astype(NP_F8)   # [128,2,256]
    pre["b2c"] = np.ascontiguousarray(b2.reshape(2, 128).T, f)        # [128,2]

    # DoubleRow ldweights needs M >= 16: pad the 2 velocity rows with zeros
    vwp = np.zeros((128, 2, 16), np.float32)
    vwp[:, :, 0:2] = (DT * W3.T).reshape(2, 128, 2).transpose(1, 0, 2)
    pre["vW8"] = vwp.astype(NP_F8)
    pre["nvW8"] = (-vwp).astype(NP_F8)

    c0, c1 = W1[:, 0], W1[:, 1]
    Wu = (W3[0][:, None] * W2) * c0[None, :] + (W3[1][:, None] * W2) * c1[None, :]
    pre["nWu8"] = np.ascontiguousarray(
        (-Wu.T).reshape(2, 128, 256).transpose(1, 0, 2)).astype(NP_F8)
    # ru row-sums of the quantized Wu, fed back on-device as a bias row in
    # the ups matmul: ups = ru - Wu @ h1sq  (true 1-h1^2 term, no host fix)
    rur = -(pre["nWu8"].astype(np.float64)).transpose(1, 0, 2) \
        .reshape(256, 256).sum(0)
    pre["rub"] = np.ascontiguousarray(rur[None, :]).astype(NP_BF16)   # [1,256]

    pre["b3c"] = np.ascontiguousarray(b3[:, None], f)                 # [2,1]
    pre["nb3c"] = np.ascontiguousarray(-b3[:, None], f)
    pre["ones2b"] = np.ones((2, N), NP_BF16)
    pre["ones120"] = np.ones((1, 20), f)

    # chamfer: bf16 compensated rows. x side fully on host.
    x = w["x"]  # not used here; per-cloud handled in kernel()
    return pre


def host_cham(xb):
    """Per-cloud chamfer aux tensors (bf16). xb: [N, 2] fp32.
    xaug:   side-A rhs rows [-2xh_x, -2xh_y, 1, 1, sqx_hi, sqx_lo].
    xplain: side-B lhsT rows [1, 1, sqx_hi, sqx_lo, xh_x, xh_y].
    """
    xh = xb.T.astype(NP_BF16)                                  # [2, N]
    xh32 = xh.astype(np.float32)
    sqx = (xh32 ** 2).sum(0)                                   # [N]
    sqx_hi = sqx.astype(NP_BF16)
    sqx_lo = (sqx - sqx_hi.astype(np.float32)).astype(NP_BF16)
    ones = np.ones((1, N), NP_BF16)
    xaug = np.concatenate([(-2.0 * xh32).astype(NP_BF16), ones, ones,
                           sqx_hi[None], sqx_lo[None]], 0)     # [6, N]
    xplain = np.concatenate([ones, ones, sqx_hi[None], sqx_lo[None], xh], 0)
    return (np.ascontiguousarray(xaug), np.ascontiguousarray(xplain))


WEIGHT_SPECS = [
    ("enc1", (3, 256), F32), ("We2T", (128, 2, 256), F32),
    ("be2r", (1, 256), F32),
    ("WmuT", (128, 2, 128), F32), ("WlvT", (128, 2, 128), F32),
    ("bmulv", (128, 2), F32),
    ("W1zT", (128, 256), F32), ("a1top", (2, 20, 2, 128), F32),
    ("TB", (20, 256), F32),
    ("W28", (128, 2, 256), F8), ("nWu8", (128, 2, 256), F8),
    ("vW8", (128, 2, 16), F8), ("nvW8", (128, 2, 16), F8),
    ("b2c", (128, 2), F32),
    ("b3c", (2, 1), F32), ("nb3c", (2, 1), F32),
    ("xaug", (6, N), BF16), ("xplain", (6, N), BF16),
    ("ones2b", (2, N), BF16), ("rub", (1, 256), BF16),
    ("ones120", (1, 20), F32),
]


def build_nc():
    nc = bacc.Bacc("TRN2", target_bir_lowering=False, debug=False,
                   enable_asserts=False, num_devices=B)
    ins = {}
    ins["xT3"] = nc.dram_tensor("xT3", [3, N], F32, kind="ExternalInput").ap()
    ins["nT3"] = nc.dram_tensor("nT3", [3, N], F32, kind="ExternalInput").ap()
    ins["epsc"] = nc.dram_tensor("epsc", [LAT, 1], F32, kind="ExternalInput").ap()
    for name, shape, dt_ in WEIGHT_SPECS:
        ins[name] = nc.dram_tensor(name, list(shape), dt_, kind="ExternalInput").ap()
    outs = {}
    for name, shape in [("o_div", [128]), ("o_mu", [128]), ("o_lv", [128]),
                        ("o_chA", [128]), ("o_chB", [128]), ("o_sy2", [2])]:
        outs[name] = nc.dram_tensor(name, shape, F32, kind="ExternalOutput").ap()

    with tile.TileContext(nc) as tc:
        _body(nc, tc, ins, outs)
    nc.compile()
    return nc


def _body(nc, tc, ins, outs):
    from contextlib import ExitStack
    with ExitStack() as ctx:
        const = ctx.enter_context(tc.tile_pool(name="const", bufs=1))
        state = ctx.enter_context(tc.tile_pool(name="state", bufs=1))
        work = ctx.enter_context(tc.tile_pool(name="work", bufs=2))
        small = ctx.enter_context(tc.tile_pool(name="small", bufs=1))

        # ---- load constants ----
        c = {}
        for name, shape, dt_ in WEIGHT_SPECS:
            if name == "a1top":
                continue  # DMA'd straight from DRAM into a1w below
            mm_names = {"enc1", "We2T", "be2r", "ones120"}
            dt2 = MDT if name in mm_names else dt_
            c[name] = const.tile(list(shape), dt2, tag=name, name=f"c_{name}")
            eng = nc.gpsimd if dt2 != dt_ else nc.sync
            eng.dma_start(out=c[name], in_=ins[name])
        ones_r = const.tile([1, NT], MDT, tag="ones_r")
        nc.gpsimd.dma_start(out=ones_r, in_=ins["xT3"][2:3, 0:NT])

        xT3 = state.tile([4, N], MDT, tag="st3", bufs=2, name="xT3")
        nc.gpsimd.dma_start(out=xT3[0:3], in_=ins["xT3"])
        # manual double-buffered euler/gen state: step i reads buf[i%2],
        # writes buf[(i+1)%2]; the ones row (row 2) is initialized once in
        # both buffers and never rewritten.
        ybuf = [[], []]
        sbuf_ = [[], []]
        for nt in range(NNT):
            sl = slice(nt * NT, (nt + 1) * NT)
            for p in range(2):
                yt = state.tile([3, NT], MDT, tag=f"y{nt}_{p}", name=f"y{nt}_{p}")
                if p == 0:
                    nc.gpsimd.dma_start(out=yt, in_=ins["xT3"][:, sl])
                else:
                    nc.gpsimd.dma_start(out=yt[2:3], in_=ins["xT3"][2:3, sl])
                ybuf[p].append(yt)
                st_ = state.tile([3, NT], MDT, tag=f"s{nt}_{p}",
                                 name=f"s{nt}_{p}")
                if p == 0:
                    nc.gpsimd.dma_start(out=st_, in_=ins["nT3"][:, sl])
                else:
                    nc.gpsimd.dma_start(out=st_[2:3], in_=ins["nT3"][2:3, sl])
                sbuf_[p].append(st_)
        eps_s = small.tile([LAT, 1], F32, tag="eps")
        nc.sync.dma_start(out=eps_s, in_=ins["epsc"])

        divacc = small.tile([128, 1], F32, tag="divacc")
        divslots = small.tile([128, STEPS * NNT], F32, tag="divslots")

        # ================= encoder =================
        g_s = small.tile([128, 2], F32, tag="g")
        gparts = small.tile([128, 2, NNT], F32, tag="gparts")
        with tc.tile_pool(name="psE", bufs=2, space="PSUM") as psE:
            for nt in range(NNT):
                sl = slice(nt * NT, (nt + 1) * NT)
                a1e = psE.tile([128, 2 * NT], F32, tag="enc")
                for mb in range(2):
                    mbs = slice(mb * 128, (mb + 1) * 128)
                    nc.tensor.matmul(a1e[:, mb * NT:(mb + 1) * NT],
                                     _mm(c["enc1"][:, mbs]), _mm(xT3[0:3, sl]),
                                     start=True, stop=True)
                h1e = work.tile([128, 2 * NT], MDT, tag="h1e")
                nc.scalar.activation(h1e, a1e, AF.Relu)
                a2e = psE.tile([128, 2 * NT], F32, tag="enc")
                for mb in range(2):
                    mbs = slice(mb * 128, (mb + 1) * 128)
                    om = a2e[:, mb * NT:(mb + 1) * NT]
                    nc.tensor.matmul(om, _mm(c["We2T"][:, 0, mbs]),
                                     _mm(h1e[:, 0:NT]), start=True, stop=False)
                    nc.tensor.matmul(om, _mm(c["We2T"][:, 1, mbs]),
                                     _mm(h1e[:, NT:2 * NT]), start=False, stop=False)
                    nc.tensor.matmul(om, _mm(c["be2r"][:, mbs]),
                                     _mm(ones_r), start=False, stop=True)
                h2e = work.tile([128, 2 * NT], MDT, tag="h2e")
                nc.scalar.activation(h2e, a2e, AF.Relu)
                for mb in range(2):
                    nc.vector.tensor_reduce(gparts[:, mb, nt:nt + 1],
                                            h2e[:, mb * NT:(mb + 1) * NT],
                                            axis=AX.X, op=ALU.max)
            for mb in range(2):
                nc.vector.tensor_reduce(g_s[:, mb:mb + 1], gparts[:, mb, :],
                                        axis=AX.X, op=ALU.max)

            mu_ps = psE.tile([128, 1], F32, tag="mu", bufs=1)
            lv_ps = psE.tile([128, 1], F32, tag="lv", bufs=1)
            for kb in range(2):
                nc.tensor.matmul(mu_ps, _mm(c["WmuT"][:, kb, :]),
                                 _mm(g_s[:, kb:kb + 1]), start=(kb == 0),
                                 stop=(kb == 1))
                nc.tensor.matmul(lv_ps, _mm(c["WlvT"][:, kb, :]),
                                 _mm(g_s[:, kb:kb + 1]), start=(kb == 0),
                                 stop=(kb == 1))
            mu_s = small.tile([128, 1], F32, tag="mu_s")
            lv_s = small.tile([128, 1], F32, tag="lv_s")
            nc.vector.tensor_scalar(mu_s, mu_ps, c["bmulv"][:, 0:1], None, ALU.add)
            nc.vector.tensor_scalar(lv_s, lv_ps, c["bmulv"][:, 1:2], None, ALU.add)
            nc.sync.dma_start(out=outs["o_mu"], in_=mu_s)
            nc.sync.dma_start(out=outs["o_lv"], in_=lv_s)
            # z = mu + eps * exp(0.5*lv)
            e_s = small.tile([128, 1], F32, tag="e_s")
            nc.scalar.activation(e_s, lv_s, AF.Exp, scale=0.5)
            z_s = small.tile([128, 1], F32, tag="z_s")
            nc.vector.tensor_tensor(z_s, e_s, eps_s, ALU.mult)
            nc.vector.tensor_tensor(z_s, z_s, mu_s, ALU.add)
            # cz_row = z @ W1zT : [1, 256]
            cz_ps = psE.tile([1, 256], F32, tag="cz", bufs=1)
            nc.tensor.matmul(cz_ps, _mm(z_s), _mm(c["W1zT"]), start=True, stop=True)
            czrow_s = small.tile([1, 256], MDT, tag="czrow_s")
            nc.vector.tensor_copy(czrow_s, cz_ps)
            # broadcast cz to 20 partitions via a K=1 ones matmul (cheaper
            # than a DRAM round-trip partition_broadcast)
            czb_ps = psE.tile([20, 256], F32, tag="czb", bufs=1)
            nc.tensor.matmul(czb_ps, _mm(c["ones120"]), _mm(czrow_s),
                             start=True, stop=True)
            brows = state.tile([20, 256], F32, tag="brows")
            nc.vector.tensor_tensor(brows, c["TB"], czb_ps, ALU.add)

        # a1w: [3, 20, 2, 128] K=3 stationary operands (W1p rows + bias row)
        a1w = state.tile([3, 20, 2, 128], MDT, tag="a1w")
        nc.gpsimd.dma_start(out=a1w[0:2], in_=ins["a1top"])
        nc.gpsimd.dma_start(out=a1w[2:3].rearrange("a b c d -> a (b c d)"),
                            in_=brows)

        # ================= CNF: euler + gen =================
        if "cnf" not in PHASES:
            return
        # Software-pipelined emission: streams processed in pairs (matching
        # the 2-slot PSUM rings); each pair's tail ops (ups/vW/scr/update)
        # are emitted during the NEXT pair's head so every engine queue has
        # ready work at its head (engines dispatch strictly in order).
        # Squares are split mb0->Pool / mb1->DVE to balance those engines.
        with tc.tile_pool(name="psA", bufs=2, space="PSUM") as psA, \
                tc.tile_pool(name="psB", bufs=2, space="PSUM") as psB:
            pairs = [(0, (0, 1)), (0, (2, 3)), (1, (0, 1)), (1, (2, 3))]
            for i in range(STEPS):
                h1sqs, upss, h2sqs, vpss = {}, {}, {}, {}
                a2s, h2s = {}, {}

                def emit_ups(nts):
                    for nt in nts:
                        h1sqv = h1sqs[nt].rearrange("p (k n) -> p k n", k=2)
                        ups = psA.tile([128, 2 * NT], F32, tag="pA", bufs=2)
                        for mb in range(2):
                            mbs = slice(mb * 128, (mb + 1) * 128)
                            om = ups[:, mb * NT:(mb + 1) * NT]
                            nc.tensor.matmul(om, _mm(c["nWu8"][:, :, mbs]),
                                             _mm(h1sqv), start=True,
                                             stop=False, perf_mode=DR)
                            nc.tensor.matmul(om, _mm(c["rub"][:, mbs]),
                                             _mm(c["ones2b"][0:1, 0:NT]),
                                             start=False, stop=True)
                        upss[nt] = ups

                def emit_tail(half, nts):
                    for nt in nts:
                        vw = c["vW8"] if half == 0 else c["nvW8"]
                        h2v = h2s[(half, nt)].rearrange("p (k n) -> p k n",
                                                        k=2)
                        vps = psB.tile([16, NT], F32, tag="pB", bufs=2)
                        nc.tensor.matmul(vps, _mm(vw), _mm(h2v),
                                         start=True, stop=True, perf_mode=DR)
                        vpss[(half, nt)] = vps
                    if half == 0:
                        for nt in nts:
                            # scr = (h2sq - 1) * ups = -s2 * t
                            # divslots col = sum(scr); host negates.
                            slot = i * NNT + nt
                            scr = work.tile([128, 2 * NT], BF16, tag="scr",
                                            bufs=3)
                            nc.vector.scalar_tensor_tensor(
                                out=scr, in0=h2sqs[nt], scalar=1.0,
                                in1=upss[nt], op0=ALU.subtract, op1=ALU.mult,
                                accum_out=divslots[:, slot:slot + 1])
                    for nt in nts:
                        stl = ybuf if half == 0 else sbuf_
                        st = stl[i % 2][nt]
                        stn = stl[(i + 1) % 2][nt]
                        nc.vector.tensor_tensor(stn[0:2, :], st[0:2, :],
                                                vpss[(half, nt)][0:2, :],
                                                ALU.add)

                prev = None
                for half, nts in pairs:
                    stl = ybuf if half == 0 else sbuf_
                    evi = i if half == 0 else STEPS + i
                    a1s = {}
                    for nt in nts:
                        st = stl[i % 2][nt]
                        a1 = psA.tile([128, 2 * NT], F32, tag="pA", bufs=2)
                        for mb in range(2):
                            nc.tensor.matmul(a1[:, mb * NT:(mb + 1) * NT],
                                             _mm(a1w[:, evi, mb, :]),
                                             _mm(st[0:3, :]),
                                             start=True, stop=True)
                        a1s[nt] = a1
                    if prev is not None and prev[0] == 0:
                        emit_ups(prev[1])
                    h1s = {}
                    for nt in nts:
                        h1 = work.tile([128, 2 * NT], F8, tag="h1", bufs=4)
                        nc.scalar.activation(h1, a1s[nt], AF.Tanh)
                        h1s[nt] = h1
                        if half == 0:
                            h1sq = work.tile([128, 2 * NT], F8, tag="h1sq",
                                             bufs=3)
                            nc.gpsimd.tensor_tensor(h1sq, h1, h1, ALU.mult)
                            h1sqs[nt] = h1sq
                    for nt in nts:
                        h1v = h1s[nt].rearrange("p (k n) -> p k n", k=2)
                        a2 = psB.tile([128, 2 * NT], F32, tag="pB", bufs=2)
                        for mb in range(2):
                            mbs = slice(mb * 128, (mb + 1) * 128)
                            nc.tensor.matmul(a2[:, mb * NT:(mb + 1) * NT],
                                             _mm(c["W28"][:, :, mbs]),
                                             _mm(h1v), start=True, stop=True,
                                             perf_mode=DR)
                        a2s[(half, nt)] = a2
                    if prev is not None:
                        emit_tail(*prev)
                    for nt in nts:
                        h2 = work.tile([128, 2 * NT], F8, tag="h2", bufs=4)
                        a2 = a2s[(half, nt)]
                        for mb in range(2):
                            ms = slice(mb * NT, (mb + 1) * NT)
                            nc.scalar.activation(h2[:, ms], a2[:, ms],
                                                 AF.Tanh,
                                                 bias=c["b2c"][:, mb:mb + 1])
                        h2s[(half, nt)] = h2
                        if half == 0:
                            h2sq = work.tile([128, 2 * NT], BF16, tag="h2sq",
                                             bufs=3)
                            nc.gpsimd.tensor_tensor(h2sq[:, 0:NT],
                                                    h2[:, 0:NT],
                                                    h2[:, 0:NT], ALU.mult)
                            nc.vector.tensor_tensor(h2sq[:, NT:], h2[:, NT:],
                                                    h2[:, NT:], ALU.mult)
                            h2sqs[nt] = h2sq
                    prev = (half, nts)
                emit_ups_last = None  # gen pairs have no ups
                emit_tail(*prev)

        nc.vector.tensor_reduce(divacc, divslots, axis=AX.X, op=ALU.add)
        nc.sync.dma_start(out=outs["o_div"], in_=divacc)

        # final y stats: y_true = y + b3 (per n-tile)
        sy2slots = small.tile([2, NNT], F32, tag="sy2slots")
        for nt in range(NNT):
            ytrue = work.tile([2, NT], F32, tag="yt", name="ytrue")
            nc.vector.tensor_scalar(ytrue, ybuf[STEPS % 2][nt][0:2, :],
                                    c["b3c"], None, ALU.add)
            sy2scr = work.tile([2, NT], F32, tag="scr2", name="sy2scr")
            nc.scalar.activation(sy2scr, ytrue, AF.Square,
                                 accum_out=sy2slots[:, nt:nt + 1])
        sy2 = small.tile([2, 1], F32, tag="sy2")
        nc.vector.tensor_reduce(sy2, sy2slots, axis=AX.X, op=ALU.add)
        nc.sync.dma_start(out=outs["o_sy2"], in_=sy2)

        # ================= chamfer =================
        if "cham" not in PHASES:
            return
        # D[p,m] = |r_p - x_m|^2 in bf16-compensated form: both clouds are
        # rounded to bf16 (smooth, unbiased shift), squared norms split
        # hi/lo so all PE products are exact in fp32 accumulation.
        # Side A (per-r min): lhsT raugA [rh_x, rh_y, sqr_hi, sqr_lo, 1, 1]
        #   x rhs xaug [-2xh_x, -2xh_y, 1, 1, sqx_hi, sqx_lo].
        # Side B (per-x min): lhsT xplain [1, 1, sqx_hi, sqx_lo, xh_x, xh_y]
        #   x rhs raugB [sqr_hi, sqr_lo, 1, 1, -2rh_x, -2rh_y].
        raugA = state.tile([6, N], BF16, tag="raugA")
        raugB = state.tile([6, N], BF16, tag="raugB")
        # engine ops need partition base in {0,32,64,96}: build rows in
        # base-0 tiles, assemble raugA/raugB via SBUF->SBUF DMA.
        rh = work.tile([2, N], BF16, tag="rh", bufs=1, name="rh")
        for nt in range(NNT):
            sl = slice(nt * NT, (nt + 1) * NT)
            nc.vector.tensor_scalar(rh[:, sl],
                                    sbuf_[STEPS % 2][nt][0:2, :],
                                    c["nb3c"], None, ALU.add)
        sqpair = work.tile([2, N], F32, tag="sqp", bufs=1, name="sqpair")
        nc.vector.scalar_tensor_tensor(out=sqpair, in0=rh, scalar=1.0,
                                       in1=rh, op0=ALU.mult, op1=ALU.mult)
        sqr = work.tile([1, N], F32, tag="sqr", bufs=1, name="sqr")
        nc.gpsimd.tensor_reduce(sqr, sqpair, axis=AX.C, op=ALU.add)
        shi = work.tile([1, N], BF16, tag="shi", bufs=1, name="shi")
        nc.vector.tensor_copy(shi, sqr)                 # sqr_hi (bf16 round)
        slo = work.tile([1, N], BF16, tag="slo", bufs=1, name="slo")
        nc.vector.tensor_tensor(slo, sqr, shi, ALU.subtract)
        nh2 = work.tile([2, N], BF16, tag="nh2", bufs=1, name="nh2")
        nc.vector.tensor_scalar(nh2, rh, -2.0, None, ALU.mult)
        nc.sync.dma_start(out=raugA[0:2], in_=rh)
        nc.sync.dma_start(out=raugA[2:3], in_=shi)
        nc.sync.dma_start(out=raugA[3:4], in_=slo)
        nc.sync.dma_start(out=raugA[4:6], in_=ins["ones2b"])
        nc.sync.dma_start(out=raugB[0:1], in_=shi)
        nc.sync.dma_start(out=raugB[1:2], in_=slo)
        nc.sync.dma_start(out=raugB[2:4], in_=ins["ones2b"])
        nc.sync.dma_start(out=raugB[4:6], in_=nh2)

        chAmin = small.tile([128, 16], F32, tag="chAmin")
        chBmin = small.tile([128, 16], F32, tag="chBmin")
        with tc.tile_pool(name="psD", bufs=2, space="PSUM") as psD:
            for blk in range(16):
                bsl = slice(blk * 128, (blk + 1) * 128)
                Dp = psD.tile([128, N], F32, tag="D")
                for mt in range(NNT):
                    msl = slice(mt * NT, (mt + 1) * NT)
                    nc.tensor.matmul(Dp[:, msl], raugA[:, bsl],
                                     c["xaug"][:, msl], start=True, stop=True)
                nc.vector.tensor_reduce(chAmin[:, blk:blk + 1], Dp,
                                        axis=AX.X, op=ALU.min)
                Dq = psD.tile([128, N], F32, tag="D")
                for mt in range(NNT):
                    msl = slice(mt * NT, (mt + 1) * NT)
                    nc.tensor.matmul(Dq[:, msl], c["xplain"][:, bsl],
                                     raugB[:, msl], start=True, stop=True)
                nc.vector.tensor_reduce(chBmin[:, blk:blk + 1], Dq,
                                        axis=AX.X, op=ALU.min)
        # clamp, sqrt, per-partition sums over the 16 blocks
        chs = small.tile([128, 16], F32, tag="chs")
        red = small.tile([128, 1], F32, tag="red")
        nc.vector.tensor_scalar_max(chAmin, chAmin, 0.0)
        nc.scalar.activation(chs, chAmin, AF.Sqrt)
        nc.vector.tensor_reduce(red, chs, axis=AX.X, op=ALU.add)
        nc.sync.dma_start(out=outs["o_chA"], in_=red)
        chs2 = small.tile([128, 16], F32, tag="chs2")
        red2 = small.tile([128, 1], F32, tag="red2")
        nc.vector.tensor_scalar_max(chBmin, chBmin, 0.0)
        nc.scalar.activation(chs2, chBmin, AF.Sqrt)
        nc.vector.tensor_reduce(red2, chs2, axis=AX.X, op=ALU.add)
        nc.sync.dma_start(out=outs["o_chB"], in_=red2)


_NC_CACHE = {}


def _get_nc():
    if "nc" not in _NC_CACHE:
        _NC_CACHE["nc"] = build_nc()
    return _NC_CACHE["nc"]


def build_in_maps(inputs, pre):
    ones_row = np.ones((1, N), np.float32)
    in_maps = []
    for b in range(B):
        m = dict(pre)
        m["xT3"] = np.ascontiguousarray(
            np.concatenate([inputs["x"][b].T, ones_row], 0), np.float32)
        m["nT3"] = np.ascontiguousarray(
            np.concatenate([inputs["noise"][b].T, ones_row], 0), np.float32)
        m["epsc"] = np.ascontiguousarray(inputs["eps"][b][:, None], np.float32)
        m["xaug"], m["xplain"] = host_cham(inputs["x"][b])
        in_maps.append(m)
    return in_maps


def kernel(**inputs):
    inputs = {k: np.asarray(v, dtype=np.float32) if np.asarray(v).dtype != np.int32
              else np.asarray(v) for k, v in inputs.items()}
    pre = host_precompute(inputs)
    nc = _get_nc()
    in_maps = build_in_maps(inputs, pre)
    res = run_bass_kernel_spmd(nc, in_maps, core_ids=list(range(B)))
    return combine(res.results, pre)


def combine(results, pre):
    S_logpy = 0.0
    S_logdet = 0.0
    prior = 0.0
    entropy = 0.0
    chamA = 0.0
    chamB = 0.0
    for r in results:
        S_logpy += -0.5 * float(r["o_sy2"].sum()) - N * LOG2PI
        # device computes divslot = sum((h2sq-1)*(ru - Wu@h1sq)) = -sum(s2*t)
        S_logdet += DT * (-float(r["o_div"].sum()))
        mu = r["o_mu"].astype(np.float64)
        lv = r["o_lv"].astype(np.float64)
        prior += 0.5 * float((mu ** 2 + np.exp(lv) - lv - 1.0).sum())
        entropy += -0.5 * float((lv + 1.0 + LOG2PI).sum())
        chamA += float(r["o_chA"].sum())
        chamB += float(r["o_chB"].sum())
    recon = -(S_logpy + S_logdet) / (B * N)
    prior /= B
    entropy /= B
    cham = chamA / (B * N) + chamB / (B * N)
    vol = max(0.0, S_logdet / (B * N) - 10.0)
    return np.float32(LAM_R * recon + LAM_P * prior + LAM_E * entropy
                      + LAM_C * cham + LAM_V * vol)


# revision 59
# speedup vs baseline: 1.0623x; 1.0623x over previous
"""PointFlow2DVAE loss kernel for 8 Trainium2 NeuronCores.

Data-parallel over batch B=8: one point cloud per core. Each core runs the
PointNet encoder, the combined Euler CNF integration (forward + generation
branch) with the exact-divergence computation folded into precomputed
matmuls, and the chamfer distance. Cores emit small partial-sum vectors;
the final scalar combine happens on host.

Perf structure:
- The two big K=256 matmuls per CNF eval (W2@h1, Wu@h1sq) and the velocity
  matmul (W3@h2) run as fp8e4m3 DoubleRow matmuls (0.5 PE cycles/row,
  K=256 folded into one call). h1/h2/h1sq are produced in fp8 directly.
- Chamfer distance uses bf16 operands with exact-product compensation:
  points are rounded to bf16 (a benign smooth shift of both clouds) and
  |r|^2/|x|^2 rows are hi/lo bf16 split so every PE product is exact in
  fp32 accumulation. Both chamfer directions are computed as separate
  matmul passes + free-axis min reductions (no PE transposes).
- Elementwise div-chain work is split across DVE (h1sq) and Pool
  (h2sq, scr) to balance engine load.
"""
import numpy as np
import ml_dtypes

import concourse.bacc as bacc
import concourse.bass as bass
import concourse.tile as tile
from concourse import mybir
from concourse.bass_utils import run_bass_kernel_spmd

B, N, D = 8, 2048, 2
LAT, ENC_H, CNF_H = 128, 256, 256
STEPS = 10
DT = 1.0 / STEPS
LAM_R, LAM_P, LAM_E, LAM_C, LAM_V = 1.0, 0.1, 0.01, 10.0, 0.01
LOG2PI = float(np.log(2.0 * np.pi))

NT = 512
NNT = N // NT
import os
PHASES = os.environ.get("KPHASES", "enc,cnf,cham").split(",")
F32 = mybir.dt.float32
MDT = mybir.dt.float32r
F8 = mybir.dt.float8e4
BF16 = mybir.dt.bfloat16
NP_F8 = mybir.dt.np(F8)
NP_BF16 = mybir.dt.np(BF16)

AF = mybir.ActivationFunctionType
ALU = mybir.AluOpType
AX = mybir.AxisListType
DR = mybir.MatmulPerfMode.DoubleRow


def _mm(ap):
    return ap


def host_precompute(w):
    f = np.float32
    W1, b1, W2, b2, W3, b3 = w["W1"], w["b1"], w["W2"], w["b2"], w["W3"], w["b3"]
    pre = {}
    pre["enc1"] = np.ascontiguousarray(
        np.concatenate([w["We1"].T, w["be1"][None, :]], 0), f)       # [3,256]
    pre["We2T"] = np.ascontiguousarray(
        w["We2"].T.reshape(2, 128, 256).transpose(1, 0, 2), f)        # [128,2,256]
    pre["be2r"] = np.ascontiguousarray(w["be2"][None, :], f)          # [1,256]
    pre["WmuT"] = np.ascontiguousarray(
        w["Wmu"].T.reshape(2, 128, 128).transpose(1, 0, 2), f)        # [128,2,128]
    pre["WlvT"] = np.ascontiguousarray(
        w["Wlv"].T.reshape(2, 128, 128).transpose(1, 0, 2), f)
    pre["bmulv"] = np.ascontiguousarray(
        np.stack([w["bmu"], w["blv"]], 1), f)                         # [128,2]

    W1p = W1[:, 0:2]
    W1t = W1[:, 2]
    pre["W1zT"] = np.ascontiguousarray(W1[:, 3:].T, f)                # [128,256]
    # a1top: W1p.T repeated for 20 evals x 2 m-blocks: [2, 20, 2, 128]
    pre["a1top"] = np.ascontiguousarray(
        np.broadcast_to(W1p.T.reshape(2, 1, 2, 128), (2, 20, 2, 128)), f)
    idx = np.arange(STEPS)
    pb3 = W1p @ b3
    TB_e = (idx * DT)[:, None] * W1t[None, :] + b1[None, :] \
        + (idx * DT)[:, None] * pb3[None, :]
    TB_g = (1.0 - idx * DT)[:, None] * W1t[None, :] + b1[None, :] \
        - (idx * DT)[:, None] * pb3[None, :]
    pre["TB"] = np.ascontiguousarray(np.concatenate([TB_e, TB_g], 0), f)  # [20,256]

    pre["W28"] = np.ascontiguousarray(
        W2.T.reshape(2, 128, 256).transpose(1, 0, 2)).astype(NP_F8)   # [128,2,256]
    pre["b2c"] = np.ascontiguousarray(b2.reshape(2, 128).T, f)        # [128,2]

    # DoubleRow ldweights needs M >= 16: pad the 2 velocity rows with zeros
    vwp = np.zeros((128, 2, 16), np.float32)
    vwp[:, :, 0:2] = (DT * W3.T).reshape(2, 128, 2).transpose(1, 0, 2)
    pre["vW8"] = vwp.astype(NP_F8)
    pre["nvW8"] = (-vwp).astype(NP_F8)

    c0, c1 = W1[:, 0], W1[:, 1]
    Wu = (W3[0][:, None] * W2) * c0[None, :] + (W3[1][:, None] * W2) * c1[None, :]
    pre["nWu8"] = np.ascontiguousarray(
        (-Wu.T).reshape(2, 128, 256).transpose(1, 0, 2)).astype(NP_F8)
    # ru row-sums of the quantized Wu, fed back on-device as a bias row in
    # the ups matmul: ups = ru - Wu @ h1sq  (true 1-h1^2 term, no host fix)
    rur = -(pre["nWu8"].astype(np.float64)).transpose(1, 0, 2) \
        .reshape(256, 256).sum(0)
    pre["rub"] = np.ascontiguousarray(rur[None, :]).astype(NP_BF16)   # [1,256]

    pre["b3c"] = np.ascontiguousarray(b3[:, None], f)                 # [2,1]
    pre["nb3c"] = np.ascontiguousarray(-b3[:, None], f)
    pre["ones2b"] = np.ones((2, N), NP_BF16)
    pre["ones120"] = np.ones((1, 20), f)

    # chamfer: bf16 compensated rows. x side fully on host.
    x = w["x"]  # not used here; per-cloud handled in kernel()
    return pre


def host_cham(xb):
    """Per-cloud chamfer aux tensors (bf16). xb: [N, 2] fp32.
    xaug:   side-A rhs rows [-2xh_x, -2xh_y, 1, 1, sqx_hi, sqx_lo].
    xplain: side-B lhsT rows [1, 1, sqx_hi, sqx_lo, xh_x, xh_y].
    """
    xh = xb.T.astype(NP_BF16)                                  # [2, N]
    xh32 = xh.astype(np.float32)
    sqx = (xh32 ** 2).sum(0)                                   # [N]
    sqx_hi = sqx.astype(NP_BF16)
    sqx_lo = (sqx - sqx_hi.astype(np.float32)).astype(NP_BF16)
    ones = np.ones((1, N), NP_BF16)
    xaug = np.concatenate([(-2.0 * xh32).astype(NP_BF16), ones, ones,
                           sqx_hi[None], sqx_lo[None]], 0)     # [6, N]
    xplain = np.concatenate([ones, ones, sqx_hi[None], sqx_lo[None], xh], 0)
    return (np.ascontiguousarray(xaug), np.ascontiguousarray(xplain))


WEIGHT_SPECS = [
    ("enc1", (3, 256), F32), ("We2T", (128, 2, 256), F32),
    ("be2r", (1, 256), F32),
    ("WmuT", (128, 2, 128), F32), ("WlvT", (128, 2, 128), F32),
    ("bmulv", (128, 2), F32),
    ("W1zT", (128, 256), F32), ("a1top", (2, 20, 2, 128), F32),
    ("TB", (20, 256), F32),
    ("W28", (128, 2, 256), F8), ("nWu8", (128, 2, 256), F8),
    ("vW8", (128, 2, 16), F8), ("nvW8", (128, 2, 16), F8),
    ("b2c", (128, 2), F32),
    ("b3c", (2, 1), F32), ("nb3c", (2, 1), F32),
    ("xaug", (6, N), BF16), ("xplain", (6, N), BF16),
    ("ones2b", (2, N), BF16), ("rub", (1, 256), BF16),
    ("ones120", (1, 20), F32),
]


def build_nc():
    nc = bacc.Bacc("TRN2", target_bir_lowering=False, debug=False,
                   enable_asserts=False, num_devices=B)
    ins = {}
    ins["xT3"] = nc.dram_tensor("xT3", [3, N], F32, kind="ExternalInput").ap()
    ins["nT3"] = nc.dram_tensor("nT3", [3, N], F32, kind="ExternalInput").ap()
    ins["epsc"] = nc.dram_tensor("epsc", [LAT, 1], F32, kind="ExternalInput").ap()
    for name, shape, dt_ in WEIGHT_SPECS:
        ins[name] = nc.dram_tensor(name, list(shape), dt_, kind="ExternalInput").ap()
    outs = {}
    for name, shape in [("o_div", [128]), ("o_mu", [128]), ("o_lv", [128]),
                        ("o_chA", [128]), ("o_chB", [128]), ("o_sy2", [2])]:
        outs[name] = nc.dram_tensor(name, shape, F32, kind="ExternalOutput").ap()

    with tile.TileContext(nc) as tc:
        _body(nc, tc, ins, outs)
    nc.compile()
    return nc


def _body(nc, tc, ins, outs):
    from contextlib import ExitStack
    with ExitStack() as ctx:
        const = ctx.enter_context(tc.tile_pool(name="const", bufs=1))
        state = ctx.enter_context(tc.tile_pool(name="state", bufs=1))
        work = ctx.enter_context(tc.tile_pool(name="work", bufs=2))
        small = ctx.enter_context(tc.tile_pool(name="small", bufs=1))

        # ---- load constants ----
        c = {}
        for name, shape, dt_ in WEIGHT_SPECS:
            if name == "a1top":
                continue  # DMA'd straight from DRAM into a1w below
            mm_names = {"enc1", "We2T", "be2r", "ones120"}
            dt2 = MDT if name in mm_names else dt_
            c[name] = const.tile(list(shape), dt2, tag=name, name=f"c_{name}")
            eng = nc.gpsimd if dt2 != dt_ else nc.sync
            eng.dma_start(out=c[name], in_=ins[name])
        ones_r = const.tile([1, NT], MDT, tag="ones_r")
        nc.gpsimd.dma_start(out=ones_r, in_=ins["xT3"][2:3, 0:NT])

        xT3 = state.tile([4, N], MDT, tag="st3", bufs=2, name="xT3")
        nc.gpsimd.dma_start(out=xT3[0:3], in_=ins["xT3"])
        # manual double-buffered euler/gen state: step i reads buf[i%2],
        # writes buf[(i+1)%2]; the ones row (row 2) is initialized once in
        # both buffers and never rewritten.
        ybuf = [[], []]
        sbuf_ = [[], []]
        for nt in range(NNT):
            sl = slice(nt * NT, (nt + 1) * NT)
            for p in range(2):
                yt = state.tile([3, NT], MDT, tag=f"y{nt}_{p}", name=f"y{nt}_{p}")
                if p == 0:
                    nc.gpsimd.dma_start(out=yt, in_=ins["xT3"][:, sl])
                else:
                    nc.gpsimd.dma_start(out=yt[2:3], in_=ins["xT3"][2:3, sl])
                ybuf[p].append(yt)
                st_ = state.tile([3, NT], MDT, tag=f"s{nt}_{p}",
                                 name=f"s{nt}_{p}")
                if p == 0:
                    nc.gpsimd.dma_start(out=st_, in_=ins["nT3"][:, sl])
                else:
                    nc.gpsimd.dma_start(out=st_[2:3], in_=ins["nT3"][2:3, sl])
                sbuf_[p].append(st_)
        eps_s = small.tile([LAT, 1], F32, tag="eps")
        nc.sync.dma_start(out=eps_s, in_=ins["epsc"])

        divacc = small.tile([128, 1], F32, tag="divacc")
        divslots = small.tile([128, STEPS * NNT], F32, tag="divslots")

        # ================= encoder =================
        g_s = small.tile([128, 2], F32, tag="g")
        gparts = small.tile([128, 2, NNT], F32, tag="gparts")
        with tc.tile_pool(name="psE", bufs=2, space="PSUM") as psE:
            for nt in range(NNT):
                sl = slice(nt * NT, (nt + 1) * NT)
                a1e = psE.tile([128, 2 * NT], F32, tag="enc")
                for mb in range(2):
                    mbs = slice(mb * 128, (mb + 1) * 128)
                    nc.tensor.matmul(a1e[:, mb * NT:(mb + 1) * NT],
                                     _mm(c["enc1"][:, mbs]), _mm(xT3[0:3, sl]),
                                     start=True, stop=True)
                h1e = work.tile([128, 2 * NT], MDT, tag="h1e")
                nc.scalar.activation(h1e, a1e, AF.Relu)
                a2e = psE.tile([128, 2 * NT], F32, tag="enc")
                for mb in range(2):
                    mbs = slice(mb * 128, (mb + 1) * 128)
                    om = a2e[:, mb * NT:(mb + 1) * NT]
                    nc.tensor.matmul(om, _mm(c["We2T"][:, 0, mbs]),
                                     _mm(h1e[:, 0:NT]), start=True, stop=False)
                    nc.tensor.matmul(om, _mm(c["We2T"][:, 1, mbs]),
                                     _mm(h1e[:, NT:2 * NT]), start=False, stop=False)
                    nc.tensor.matmul(om, _mm(c["be2r"][:, mbs]),
                                     _mm(ones_r), start=False, stop=True)
                h2e = work.tile([128, 2 * NT], MDT, tag="h2e")
                nc.scalar.activation(h2e, a2e, AF.Relu)
                for mb in range(2):
                    nc.vector.tensor_reduce(gparts[:, mb, nt:nt + 1],
                                            h2e[:, mb * NT:(mb + 1) * NT],
                                            axis=AX.X, op=ALU.max)
            for mb in range(2):
                nc.vector.tensor_reduce(g_s[:, mb:mb + 1], gparts[:, mb, :],
                                        axis=AX.X, op=ALU.max)

            mu_ps = psE.tile([128, 1], F32, tag="mu", bufs=1)
            lv_ps = psE.tile([128, 1], F32, tag="lv", bufs=1)
            for kb in range(2):
                nc.tensor.matmul(mu_ps, _mm(c["WmuT"][:, kb, :]),
                                 _mm(g_s[:, kb:kb + 1]), start=(kb == 0),
                                 stop=(kb == 1))
                nc.tensor.matmul(lv_ps, _mm(c["WlvT"][:, kb, :]),
                                 _mm(g_s[:, kb:kb + 1]), start=(kb == 0),
                                 stop=(kb == 1))
            mu_s = small.tile([128, 1], F32, tag="mu_s")
            lv_s = small.tile([128, 1], F32, tag="lv_s")
            nc.vector.tensor_scalar(mu_s, mu_ps, c["bmulv"][:, 0:1], None, ALU.add)
            nc.vector.tensor_scalar(lv_s, lv_ps, c["bmulv"][:, 1:2], None, ALU.add)
            nc.sync.dma_start(out=outs["o_mu"], in_=mu_s)
            nc.sync.dma_start(out=outs["o_lv"], in_=lv_s)
            # z = mu + eps * exp(0.5*lv)
            e_s = small.tile([128, 1], F32, tag="e_s")
            nc.scalar.activation(e_s, lv_s, AF.Exp, scale=0.5)
            z_s = small.tile([128, 1], F32, tag="z_s")
            nc.vector.tensor_tensor(z_s, e_s, eps_s, ALU.mult)
            nc.vector.tensor_tensor(z_s, z_s, mu_s, ALU.add)
            # cz_row = z @ W1zT : [1, 256]
            cz_ps = psE.tile([1, 256], F32, tag="cz", bufs=1)
            nc.tensor.matmul(cz_ps, _mm(z_s), _mm(c["W1zT"]), start=True, stop=True)
            czrow_s = small.tile([1, 256], MDT, tag="czrow_s")
            nc.vector.tensor_copy(czrow_s, cz_ps)
            # broadcast cz to 20 partitions via a K=1 ones matmul (cheaper
            # than a DRAM round-trip partition_broadcast)
            czb_ps = psE.tile([20, 256], F32, tag="czb", bufs=1)
            nc.tensor.matmul(czb_ps, _mm(c["ones120"]), _mm(czrow_s),
                             start=True, stop=True)
            brows = state.tile([20, 256], F32, tag="brows")
            nc.vector.tensor_tensor(brows, c["TB"], czb_ps, ALU.add)

        # a1w: [3, 20, 2, 128] K=3 stationary operands (W1p rows + bias row)
        a1w = state.tile([3, 20, 2, 128], MDT, tag="a1w")
        nc.gpsimd.dma_start(out=a1w[0:2], in_=ins["a1top"])
        nc.gpsimd.dma_start(out=a1w[2:3].rearrange("a b c d -> a (b c d)"),
                            in_=brows)

        # ================= CNF: euler + gen =================
        if "cnf" not in PHASES:
            return
        # Software-pipelined emission: streams processed in pairs (matching
        # the 2-slot PSUM rings); each pair's tail ops (ups/vW/scr/update)
        # are emitted during the NEXT pair's head so every engine queue has
        # ready work at its head (engines dispatch strictly in order).
        # Squares are split mb0->Pool / mb1->DVE to balance those engines.
        with tc.tile_pool(name="psA", bufs=2, space="PSUM") as psA, \
                tc.tile_pool(name="psB", bufs=2, space="PSUM") as psB:
            pairs = [(0, (0, 1)), (0, (2, 3)), (1, (0, 1)), (1, (2, 3))]
            for i in range(STEPS):
                h1sqs, upss, h2sqs, vpss = {}, {}, {}, {}
                a2s, h2s = {}, {}

                def emit_ups(nts):
                    for nt in nts:
                        h1sqv = h1sqs[nt].rearrange("p (k n) -> p k n", k=2)
                        ups = psA.tile([128, 2 * NT], F32, tag="pA", bufs=2)
                        for mb in range(2):
                            mbs = slice(mb * 128, (mb + 1) * 128)
                            om = ups[:, mb * NT:(mb + 1) * NT]
                            nc.tensor.matmul(om, _mm(c["nWu8"][:, :, mbs]),
                                             _mm(h1sqv), start=True,
                                             stop=False, perf_mode=DR)
                            nc.tensor.matmul(om, _mm(c["rub"][:, mbs]),
                                             _mm(c["ones2b"][0:1, 0:NT]),
                                             start=False, stop=True)
                        upss[nt] = ups

                def emit_tail(half, nts):
                    for nt in nts:
                        vw = c["vW8"] if half == 0 else c["nvW8"]
                        h2v = h2s[(half, nt)].rearrange("p (k n) -> p k n",
                                                        k=2)
                        vps = psB.tile([16, NT], F32, tag="pB", bufs=2)
                        nc.tensor.matmul(vps, _mm(vw), _mm(h2v),
                                         start=True, stop=True, perf_mode=DR)
                        vpss[(half, nt)] = vps
                    if half == 0:
                        for nt in nts:
                            # scr = (h2sq - 1) * ups = -s2 * t
                            # divslots col = sum(scr); host negates.
                            slot = i * NNT + nt
                            scr = work.tile([128, 2 * NT], BF16, tag="scr",
                                            bufs=3)
                            nc.vector.scalar_tensor_tensor(
                                out=scr, in0=h2sqs[nt], scalar=1.0,
                                in1=upss[nt], op0=ALU.subtract, op1=ALU.mult,
                                accum_out=divslots[:, slot:slot + 1])
                    for nt in nts:
                        stl = ybuf if half == 0 else sbuf_
                        st = stl[i % 2][nt]
                        stn = stl[(i + 1) % 2][nt]
                        nc.vector.tensor_tensor(stn[0:2, :], st[0:2, :],
                                                vpss[(half, nt)][0:2, :],
                                                ALU.add)

                prev = None
                for half, nts in pairs:
                    stl = ybuf if half == 0 else sbuf_
                    evi = i if half == 0 else STEPS + i
                    a1s = {}
                    for nt in nts:
                        st = stl[i % 2][nt]
                        a1 = psA.tile([128, 2 * NT], F32, tag="pA", bufs=2)
                        for mb in range(2):
                            nc.tensor.matmul(a1[:, mb * NT:(mb + 1) * NT],
                                             _mm(a1w[:, evi, mb, :]),
                                             _mm(st[0:3, :]),
                                             start=True, stop=True)
                        a1s[nt] = a1
                    if prev is not None and prev[0] == 0:
                        emit_ups(prev[1])
                    h1s = {}
                    for nt in nts:
                        h1 = work.tile([128, 2 * NT], F8, tag="h1", bufs=4)
                        nc.scalar.activation(h1, a1s[nt], AF.Tanh)
                        h1s[nt] = h1
                        if half == 0:
                            h1sq = work.tile([128, 2 * NT], F8, tag="h1sq",
                                             bufs=3)
                            nc.gpsimd.tensor_tensor(h1sq[:, 0:NT],
                                                    h1[:, 0:NT],
                                                    h1[:, 0:NT], ALU.mult)
                            nc.vector.tensor_tensor(h1sq[:, NT:], h1[:, NT:],
                                                    h1[:, NT:], ALU.mult)
                            h1sqs[nt] = h1sq
                    for nt in nts:
                        h1v = h1s[nt].rearrange("p (k n) -> p k n", k=2)
                        a2 = psB.tile([128, 2 * NT], F32, tag="pB", bufs=2)
                        for mb in range(2):
                            mbs = slice(mb * 128, (mb + 1) * 128)
                            nc.tensor.matmul(a2[:, mb * NT:(mb + 1) * NT],
                                             _mm(c["W28"][:, :, mbs]),
                                             _mm(h1v), start=True, stop=True,
                                             perf_mode=DR)
                        a2s[(half, nt)] = a2
                    if prev is not None:
                        emit_tail(*prev)
                    for nt in nts:
                        h2 = work.tile([128, 2 * NT], F8, tag="h2", bufs=4)
                        a2 = a2s[(half, nt)]
                        for mb in range(2):
                            ms = slice(mb * NT, (mb + 1) * NT)
                            nc.scalar.activation(h2[:, ms], a2[:, ms],
                                                 AF.Tanh,
                                                 bias=c["b2c"][:, mb:mb + 1])
                        h2s[(half, nt)] = h2
                        if half == 0:
                            h2sq = work.tile([128, 2 * NT], BF16, tag="h2sq",
                                             bufs=3)
                            nc.gpsimd.tensor_tensor(h2sq[:, 0:NT],
                                                    h2[:, 0:NT],
                                                    h2[:, 0:NT], ALU.mult)
                            nc.vector.tensor_tensor(h2sq[:, NT:], h2[:, NT:],
                                                    h2[:, NT:], ALU.mult)
                            h2sqs[nt] = h2sq
                    prev = (half, nts)
                emit_ups_last = None  # gen pairs have no ups
                emit_tail(*prev)

        nc.vector.tensor_reduce(divacc, divslots, axis=AX.X, op=ALU.add)
        nc.sync.dma_start(out=outs["o_div"], in_=divacc)

        # final y stats: y_true = y + b3 (per n-tile)
        sy2slots = small.tile([2, NNT], F32, tag="sy2slots")
        for nt in range(NNT):
            ytrue = work.tile([2, NT], F32, tag="yt", name="ytrue")
            nc.vector.tensor_scalar(ytrue, ybuf[STEPS % 2][nt][0:2, :],
                                    c["b3c"], None, ALU.add)
            sy2scr = work.tile([2, NT], F32, tag="scr2", name="sy2scr")
            nc.scalar.activation(sy2scr, ytrue, AF.Square,
                                 accum_out=sy2slots[:, nt:nt + 1])
        sy2 = small.tile([2, 1], F32, tag="sy2")
        nc.vector.tensor_reduce(sy2, sy2slots, axis=AX.X, op=ALU.add)
        nc.sync.dma_start(out=outs["o_sy2"], in_=sy2)

        # ================= chamfer =================
        if "cham" not in PHASES:
            return
        # D[p,m] = |r_p - x_m|^2 in bf16-compensated form: both clouds are
        # rounded to bf16 (smooth, unbiased shift), squared norms split
        # hi/lo so all PE products are exact in fp32 accumulation.
        # Side A (per-r min): lhsT raugA [rh_x, rh_y, sqr_hi, sqr_lo, 1, 1]
        #   x rhs xaug [-2xh_x, -2xh_y, 1, 1, sqx_hi, sqx_lo].
        # Side B (per-x min): lhsT xplain [1, 1, sqx_hi, sqx_lo, xh_x, xh_y]
        #   x rhs raugB [sqr_hi, sqr_lo, 1, 1, -2rh_x, -2rh_y].
        raugA = state.tile([6, N], BF16, tag="raugA")
        raugB = state.tile([6, N], BF16, tag="raugB")
        # engine ops need partition base in {0,32,64,96}: build rows in
        # base-0 tiles, assemble raugA/raugB via SBUF->SBUF DMA.
        rh = work.tile([2, N], BF16, tag="rh", bufs=1, name="rh")
        for nt in range(NNT):
            sl = slice(nt * NT, (nt + 1) * NT)
            nc.vector.tensor_scalar(rh[:, sl],
                                    sbuf_[STEPS % 2][nt][0:2, :],
                                    c["nb3c"], None, ALU.add)
        sqpair = work.tile([2, N], F32, tag="sqp", bufs=1, name="sqpair")
        nc.vector.scalar_tensor_tensor(out=sqpair, in0=rh, scalar=1.0,
                                       in1=rh, op0=ALU.mult, op1=ALU.mult)
        sqr = work.tile([1, N], F32, tag="sqr", bufs=1, name="sqr")
        nc.gpsimd.tensor_reduce(sqr, sqpair, axis=AX.C, op=ALU.add)
        shi = work.tile([1, N], BF16, tag="shi", bufs=1, name="shi")
        nc.vector.tensor_copy(shi, sqr)                 # sqr_hi (bf16 round)
        slo = work.tile([1, N], BF16, tag="slo", bufs=1, name="slo")
        nc.vector.tensor_tensor(slo, sqr, shi, ALU.subtract)
        nh2 = work.tile([2, N], BF16, tag="nh2", bufs=1, name="nh2")
        nc.vector.tensor_scalar(nh2, rh, -2.0, None, ALU.mult)
        nc.sync.dma_start(out=raugA[0:2], in_=rh)
        nc.sync.dma_start(out=raugA[2:3], in_=shi)
        nc.sync.dma_start(out=raugA[3:4], in_=slo)
        nc.sync.dma_start(out=raugA[4:6], in_=ins["ones2b"])
        nc.sync.dma_start(out=raugB[0:1], in_=shi)
        nc.sync.dma_start(out=raugB[1:2], in_=slo)
        nc.sync.dma_start(out=raugB[2:4], in_=ins["ones2b"])
        nc.sync.dma_start(out=raugB[4:6], in_=nh2)

        chAmin = small.tile([128, 16], F32, tag="chAmin")
        chBmin = small.tile([128, 16], F32, tag="chBmin")
        with tc.tile_pool(name="psD", bufs=2, space="PSUM") as psD:
            for blk in range(16):
                bsl = slice(blk * 128, (blk + 1) * 128)
                Dp = psD.tile([128, N], F32, tag="D")
                for mt in range(NNT):
                    msl = slice(mt * NT, (mt + 1) * NT)
                    nc.tensor.matmul(Dp[:, msl], raugA[:, bsl],
                                     c["xaug"][:, msl], start=True, stop=True)
                nc.vector.tensor_reduce(chAmin[:, blk:blk + 1], Dp,
                                        axis=AX.X, op=ALU.min)
                Dq = psD.tile([128, N], F32, tag="D")
                for mt in range(NNT):
                    msl = slice(mt * NT, (mt + 1) * NT)
                    nc.tensor.matmul(Dq[:, msl], c["xplain"][:, bsl],
                                     raugB[:, msl], start=True, stop=True)
                nc.vector.tensor_reduce(chBmin[:, blk:blk + 1], Dq,
                                        axis=AX.X, op=ALU.min)
        # clamp, sqrt, per-partition sums over the 16 blocks
        chs = small.tile([128, 16], F32, tag="chs")
        red = small.tile([128, 1], F32, tag="red")
        nc.vector.tensor_scalar_max(chAmin, chAmin, 0.0)
        nc.scalar.activation(chs, chAmin, AF.Sqrt)
        nc.vector.tensor_reduce(red, chs, axis=AX.X, op=ALU.add)
        nc.sync.dma_start(out=outs["o_chA"], in_=red)
        chs2 = small.tile([128, 16], F32, tag="chs2")
        red2 = small.tile([128, 1], F32, tag="red2")
        nc.vector.tensor_scalar_max(chBmin, chBmin, 0.0)
        nc.scalar.activation(chs2, chBmin, AF.Sqrt)
        nc.vector.tensor_reduce(red2, chs2, axis=AX.X, op=ALU.add)
        nc.sync.dma_start(out=outs["o_chB"], in_=red2)


_NC_CACHE = {}


def _get_nc():
    if "nc" not in _NC_CACHE:
        _NC_CACHE["nc"] = build_nc()
    return _NC_CACHE["nc"]


def build_in_maps(inputs, pre):
    ones_row = np.ones((1, N), np.float32)
    in_maps = []
    for b in range(B):
        m = dict(pre)
        m["xT3"] = np.ascontiguousarray(
            np.concatenate([inputs["x"][b].T, ones_row], 0), np.float32)
        m["nT3"] = np.ascontiguousarray(
            np.concatenate([inputs["noise"][b].T, ones_row], 0), np.float32)
        m["epsc"] = np.ascontiguousarray(inputs["eps"][b][:, None], np.float32)
        m["xaug"], m["xplain"] = host_cham(inputs["x"][b])
        in_maps.append(m)
    return in_maps


def kernel(**inputs):
    inputs = {k: np.asarray(v, dtype=np.float32) if np.asarray(v).dtype != np.int32
              else np.asarray(v) for k, v in inputs.items()}
    pre = host_precompute(inputs)
    nc = _get_nc()
    in_maps = build_in_maps(inputs, pre)
    res = run_bass_kernel_spmd(nc, in_maps, core_ids=list(range(B)))
    return combine(res.results, pre)


def combine(results, pre):
    S_logpy = 0.0
    S_logdet = 0.0
    prior = 0.0
    entropy = 0.0
    chamA = 0.0
    chamB = 0.0
    for r in results:
        S_logpy += -0.5 * float(r["o_sy2"].sum()) - N * LOG2PI
        # device computes divslot = sum((h2sq-1)*(ru - Wu@h1sq)) = -sum(s2*t)
        S_logdet += DT * (-float(r["o_div"].sum()))
        mu = r["o_mu"].astype(np.float64)
        lv = r["o_lv"].astype(np.float64)
        prior += 0.5 * float((mu ** 2 + np.exp(lv) - lv - 1.0).sum())
        entropy += -0.5 * float((lv + 1.0 + LOG2PI).sum())
        chamA += float(r["o_chA"].sum())
        chamB += float(r["o_chB"].sum())
    recon = -(S_logpy + S_logdet) / (B * N)
    prior /= B
    entropy /= B
    cham = chamA / (B * N) + chamB / (B * N)
    vol = max(0.0, S_logdet / (B * N) - 10.0)
    return np.float32(LAM_R * recon + LAM_P * prior + LAM_E * entropy
                      + LAM_C * cham + LAM_V * vol)


# revision 60
# speedup vs baseline: 1.0628x; 1.0004x over previous
"""PointFlow2DVAE loss kernel for 8 Trainium2 NeuronCores.

Data-parallel over batch B=8: one point cloud per core. Each core runs the
PointNet encoder, the combined Euler CNF integration (forward + generation
branch) with the exact-divergence computation folded into precomputed
matmuls, and the chamfer distance. Cores emit small partial-sum vectors;
the final scalar combine happens on host.

Perf structure:
- The two big K=256 matmuls per CNF eval (W2@h1, Wu@h1sq) and the velocity
  matmul (W3@h2) run as fp8e4m3 DoubleRow matmuls (0.5 PE cycles/row,
  K=256 folded into one call). h1/h2/h1sq are produced in fp8 directly.
- Chamfer distance uses bf16 operands with exact-product compensation:
  points are rounded to bf16 (a benign smooth shift of both clouds) and
  |r|^2/|x|^2 rows are hi/lo bf16 split so every PE product is exact in
  fp32 accumulation. Both chamfer directions are computed as separate
  matmul passes + free-axis min reductions (no PE transposes).
- Elementwise div-chain work is split across DVE (h1sq) and Pool
  (h2sq, scr) to balance engine load.
"""
import numpy as np
import ml_dtypes

import concourse.bacc as bacc
import concourse.bass as bass
import concourse.tile as tile
from concourse import mybir
from concourse.bass_utils import run_bass_kernel_spmd

B, N, D = 8, 2048, 2
LAT, ENC_H, CNF_H = 128, 256, 256
STEPS = 10
DT = 1.0 / STEPS
LAM_R, LAM_P, LAM_E, LAM_C, LAM_V = 1.0, 0.1, 0.01, 10.0, 0.01
LOG2PI = float(np.log(2.0 * np.pi))

NT = 512
NNT = N // NT
import os
PHASES = os.environ.get("KPHASES", "enc,cnf,cham").split(",")
F32 = mybir.dt.float32
MDT = mybir.dt.float32r
F8 = mybir.dt.float8e4
BF16 = mybir.dt.bfloat16
NP_F8 = mybir.dt.np(F8)
NP_BF16 = mybir.dt.np(BF16)

AF = mybir.ActivationFunctionType
ALU = mybir.AluOpType
AX = mybir.AxisListType
DR = mybir.MatmulPerfMode.DoubleRow


def _mm(ap):
    return ap


def host_precompute(w):
    f = np.float32
    W1, b1, W2, b2, W3, b3 = w["W1"], w["b1"], w["W2"], w["b2"], w["W3"], w["b3"]
    pre = {}
    pre["enc1"] = np.ascontiguousarray(
        np.concatenate([w["We1"].T, w["be1"][None, :]], 0), f)       # [3,256]
    pre["We2T"] = np.ascontiguousarray(
        w["We2"].T.reshape(2, 128, 256).transpose(1, 0, 2), f)        # [128,2,256]
    pre["be2r"] = np.ascontiguousarray(w["be2"][None, :], f)          # [1,256]
    pre["WmuT"] = np.ascontiguousarray(
        w["Wmu"].T.reshape(2, 128, 128).transpose(1, 0, 2), f)        # [128,2,128]
    pre["WlvT"] = np.ascontiguousarray(
        w["Wlv"].T.reshape(2, 128, 128).transpose(1, 0, 2), f)
    pre["bmulv"] = np.ascontiguousarray(
        np.stack([w["bmu"], w["blv"]], 1), f)                         # [128,2]

    W1p = W1[:, 0:2]
    W1t = W1[:, 2]
    pre["W1zT"] = np.ascontiguousarray(W1[:, 3:].T, f)                # [128,256]
    # a1top: W1p.T repeated for 20 evals x 2 m-blocks: [2, 20, 2, 128]
    pre["a1top"] = np.ascontiguousarray(
        np.broadcast_to(W1p.T.reshape(2, 1, 2, 128), (2, 20, 2, 128)), f)
    idx = np.arange(STEPS)
    pb3 = W1p @ b3
    TB_e = (idx * DT)[:, None] * W1t[None, :] + b1[None, :] \
        + (idx * DT)[:, None] * pb3[None, :]
    TB_g = (1.0 - idx * DT)[:, None] * W1t[None, :] + b1[None, :] \
        - (idx * DT)[:, None] * pb3[None, :]
    pre["TB"] = np.ascontiguousarray(np.concatenate([TB_e, TB_g], 0), f)  # [20,256]

    pre["W28"] = np.ascontiguousarray(
        W2.T.reshape(2, 128, 256).transpose(1, 0, 2)).astype(NP_F8)   # [128,2,256]
    pre["b2c"] = np.ascontiguousarray(b2.reshape(2, 128).T, f)        # [128,2]

    # DoubleRow ldweights needs M >= 16: pad the 2 velocity rows with zeros
    vwp = np.zeros((128, 2, 16), np.float32)
    vwp[:, :, 0:2] = (DT * W3.T).reshape(2, 128, 2).transpose(1, 0, 2)
    pre["vW8"] = vwp.astype(NP_F8)
    pre["nvW8"] = (-vwp).astype(NP_F8)

    c0, c1 = W1[:, 0], W1[:, 1]
    Wu = (W3[0][:, None] * W2) * c0[None, :] + (W3[1][:, None] * W2) * c1[None, :]
    pre["nWu8"] = np.ascontiguousarray(
        (-Wu.T).reshape(2, 128, 256).transpose(1, 0, 2)).astype(NP_F8)
    # ru row-sums of the quantized Wu, fed back on-device as a bias row in
    # the ups matmul: ups = ru - Wu @ h1sq  (true 1-h1^2 term, no host fix)
    rur = -(pre["nWu8"].astype(np.float64)).transpose(1, 0, 2) \
        .reshape(256, 256).sum(0)
    pre["rub"] = np.ascontiguousarray(rur[None, :]).astype(NP_BF16)   # [1,256]

    pre["b3c"] = np.ascontiguousarray(b3[:, None], f)                 # [2,1]
    pre["nb3c"] = np.ascontiguousarray(-b3[:, None], f)
    pre["ones2b"] = np.ones((2, N), NP_BF16)
    pre["ones120"] = np.ones((1, 20), f)

    # chamfer: bf16 compensated rows. x side fully on host.
    x = w["x"]  # not used here; per-cloud handled in kernel()
    return pre


def host_cham(xb):
    """Per-cloud chamfer aux tensors (bf16). xb: [N, 2] fp32.
    xaug:   side-A rhs rows [-2xh_x, -2xh_y, 1, 1, sqx_hi, sqx_lo].
    xplain: side-B lhsT rows [1, 1, sqx_hi, sqx_lo, xh_x, xh_y].
    """
    xh = xb.T.astype(NP_BF16)                                  # [2, N]
    xh32 = xh.astype(np.float32)
    sqx = (xh32 ** 2).sum(0)                                   # [N]
    sqx_hi = sqx.astype(NP_BF16)
    sqx_lo = (sqx - sqx_hi.astype(np.float32)).astype(NP_BF16)
    ones = np.ones((1, N), NP_BF16)
    xaug = np.concatenate([(-2.0 * xh32).astype(NP_BF16), ones, ones,
                           sqx_hi[None], sqx_lo[None]], 0)     # [6, N]
    xplain = np.concatenate([ones, ones, sqx_hi[None], sqx_lo[None], xh], 0)
    return (np.ascontiguousarray(xaug), np.ascontiguousarray(xplain))


WEIGHT_SPECS = [
    ("enc1", (3, 256), F32), ("We2T", (128, 2, 256), F32),
    ("be2r", (1, 256), F32),
    ("WmuT", (128, 2, 128), F32), ("WlvT", (128, 2, 128), F32),
    ("bmulv", (128, 2), F32),
    ("W1zT", (128, 256), F32), ("a1top", (2, 20, 2, 128), F32),
    ("TB", (20, 256), F32),
    ("W28", (128, 2, 256), F8), ("nWu8", (128, 2, 256), F8),
    ("vW8", (128, 2, 16), F8), ("nvW8", (128, 2, 16), F8),
    ("b2c", (128, 2), F32),
    ("b3c", (2, 1), F32), ("nb3c", (2, 1), F32),
    ("xaug", (6, N), BF16), ("xplain", (6, N), BF16),
    ("ones2b", (2, N), BF16), ("rub", (1, 256), BF16),
    ("ones120", (1, 20), F32),
]


def build_nc():
    nc = bacc.Bacc("TRN2", target_bir_lowering=False, debug=False,
                   enable_asserts=False, num_devices=B)
    ins = {}
    ins["xT3"] = nc.dram_tensor("xT3", [3, N], F32, kind="ExternalInput").ap()
    ins["nT3"] = nc.dram_tensor("nT3", [3, N], F32, kind="ExternalInput").ap()
    ins["epsc"] = nc.dram_tensor("epsc", [LAT, 1], F32, kind="ExternalInput").ap()
    for name, shape, dt_ in WEIGHT_SPECS:
        ins[name] = nc.dram_tensor(name, list(shape), dt_, kind="ExternalInput").ap()
    outs = {}
    for name, shape in [("o_div", [128]), ("o_mu", [128]), ("o_lv", [128]),
                        ("o_chA", [128]), ("o_chB", [128]), ("o_sy2", [2])]:
        outs[name] = nc.dram_tensor(name, shape, F32, kind="ExternalOutput").ap()

    with tile.TileContext(nc) as tc:
        _body(nc, tc, ins, outs)
    nc.compile()
    return nc


def _body(nc, tc, ins, outs):
    from contextlib import ExitStack
    with ExitStack() as ctx:
        const = ctx.enter_context(tc.tile_pool(name="const", bufs=1))
        state = ctx.enter_context(tc.tile_pool(name="state", bufs=1))
        work = ctx.enter_context(tc.tile_pool(name="work", bufs=2))
        small = ctx.enter_context(tc.tile_pool(name="small", bufs=1))

        # ---- load constants ----
        c = {}
        for name, shape, dt_ in WEIGHT_SPECS:
            if name == "a1top":
                continue  # DMA'd straight from DRAM into a1w below
            mm_names = {"enc1", "We2T", "be2r", "ones120"}
            dt2 = MDT if name in mm_names else dt_
            c[name] = const.tile(list(shape), dt2, tag=name, name=f"c_{name}")
            eng = nc.gpsimd if dt2 != dt_ else nc.sync
            eng.dma_start(out=c[name], in_=ins[name])
        ones_r = const.tile([1, NT], MDT, tag="ones_r")
        nc.gpsimd.dma_start(out=ones_r, in_=ins["xT3"][2:3, 0:NT])

        xT3 = state.tile([4, N], MDT, tag="st3", bufs=2, name="xT3")
        nc.gpsimd.dma_start(out=xT3[0:3], in_=ins["xT3"])
        # manual double-buffered euler/gen state: step i reads buf[i%2],
        # writes buf[(i+1)%2]; the ones row (row 2) is initialized once in
        # both buffers and never rewritten.
        ybuf = [[], []]
        sbuf_ = [[], []]
        for nt in range(NNT):
            sl = slice(nt * NT, (nt + 1) * NT)
            for p in range(2):
                yt = state.tile([3, NT], MDT, tag=f"y{nt}_{p}", name=f"y{nt}_{p}")
                if p == 0:
                    nc.gpsimd.dma_start(out=yt, in_=ins["xT3"][:, sl])
                else:
                    nc.gpsimd.dma_start(out=yt[2:3], in_=ins["xT3"][2:3, sl])
                ybuf[p].append(yt)
                st_ = state.tile([3, NT], MDT, tag=f"s{nt}_{p}",
                                 name=f"s{nt}_{p}")
                if p == 0:
                    nc.gpsimd.dma_start(out=st_, in_=ins["nT3"][:, sl])
                else:
                    nc.gpsimd.dma_start(out=st_[2:3], in_=ins["nT3"][2:3, sl])
                sbuf_[p].append(st_)
        eps_s = small.tile([LAT, 1], F32, tag="eps")
        nc.sync.dma_start(out=eps_s, in_=ins["epsc"])

        divacc = small.tile([128, 1], F32, tag="divacc")
        divslots = small.tile([128, STEPS * NNT], F32, tag="divslots")

        # ================= encoder =================
        g_s = small.tile([128, 2], F32, tag="g")
        gparts = small.tile([128, 2, NNT], F32, tag="gparts")
        with tc.tile_pool(name="psE", bufs=2, space="PSUM") as psE:
            for nt in range(NNT):
                sl = slice(nt * NT, (nt + 1) * NT)
                a1e = psE.tile([128, 2 * NT], F32, tag="enc")
                for mb in range(2):
                    mbs = slice(mb * 128, (mb + 1) * 128)
                    nc.tensor.matmul(a1e[:, mb * NT:(mb + 1) * NT],
                                     _mm(c["enc1"][:, mbs]), _mm(xT3[0:3, sl]),
                                     start=True, stop=True)
                h1e = work.tile([128, 2 * NT], MDT, tag="h1e")
                nc.scalar.activation(h1e, a1e, AF.Relu)
                a2e = psE.tile([128, 2 * NT], F32, tag="enc")
                for mb in range(2):
                    mbs = slice(mb * 128, (mb + 1) * 128)
                    om = a2e[:, mb * NT:(mb + 1) * NT]
                    nc.tensor.matmul(om, _mm(c["We2T"][:, 0, mbs]),
                                     _mm(h1e[:, 0:NT]), start=True, stop=False)
                    nc.tensor.matmul(om, _mm(c["We2T"][:, 1, mbs]),
                                     _mm(h1e[:, NT:2 * NT]), start=False, stop=False)
                    nc.tensor.matmul(om, _mm(c["be2r"][:, mbs]),
                                     _mm(ones_r), start=False, stop=True)
                h2e = work.tile([128, 2 * NT], MDT, tag="h2e")
                nc.scalar.activation(h2e, a2e, AF.Relu)
                for mb in range(2):
                    nc.vector.tensor_reduce(gparts[:, mb, nt:nt + 1],
                                            h2e[:, mb * NT:(mb + 1) * NT],
                                            axis=AX.X, op=ALU.max)
            for mb in range(2):
                nc.vector.tensor_reduce(g_s[:, mb:mb + 1], gparts[:, mb, :],
                                        axis=AX.X, op=ALU.max)

            mu_ps = psE.tile([128, 1], F32, tag="mu", bufs=1)
            lv_ps = psE.tile([128, 1], F32, tag="lv", bufs=1)
            for kb in range(2):
                nc.tensor.matmul(mu_ps, _mm(c["WmuT"][:, kb, :]),
                                 _mm(g_s[:, kb:kb + 1]), start=(kb == 0),
                                 stop=(kb == 1))
                nc.tensor.matmul(lv_ps, _mm(c["WlvT"][:, kb, :]),
                                 _mm(g_s[:, kb:kb + 1]), start=(kb == 0),
                                 stop=(kb == 1))
            mu_s = small.tile([128, 1], F32, tag="mu_s")
            lv_s = small.tile([128, 1], F32, tag="lv_s")
            nc.vector.tensor_scalar(mu_s, mu_ps, c["bmulv"][:, 0:1], None, ALU.add)
            nc.vector.tensor_scalar(lv_s, lv_ps, c["bmulv"][:, 1:2], None, ALU.add)
            nc.sync.dma_start(out=outs["o_mu"], in_=mu_s)
            nc.sync.dma_start(out=outs["o_lv"], in_=lv_s)
            # z = mu + eps * exp(0.5*lv)
            e_s = small.tile([128, 1], F32, tag="e_s")
            nc.scalar.activation(e_s, lv_s, AF.Exp, scale=0.5)
            z_s = small.tile([128, 1], F32, tag="z_s")
            nc.vector.tensor_tensor(z_s, e_s, eps_s, ALU.mult)
            nc.vector.tensor_tensor(z_s, z_s, mu_s, ALU.add)
            # cz_row = z @ W1zT : [1, 256]
            cz_ps = psE.tile([1, 256], F32, tag="cz", bufs=1)
            nc.tensor.matmul(cz_ps, _mm(z_s), _mm(c["W1zT"]), start=True, stop=True)
            czrow_s = small.tile([1, 256], MDT, tag="czrow_s")
            nc.vector.tensor_copy(czrow_s, cz_ps)
            # broadcast cz to 20 partitions via a K=1 ones matmul (cheaper
            # than a DRAM round-trip partition_broadcast)
            czb_ps = psE.tile([20, 256], F32, tag="czb", bufs=1)
            nc.tensor.matmul(czb_ps, _mm(c["ones120"]), _mm(czrow_s),
                             start=True, stop=True)
            brows = state.tile([20, 256], F32, tag="brows")
            nc.vector.tensor_tensor(brows, c["TB"], czb_ps, ALU.add)

        # a1w: [3, 20, 2, 128] K=3 stationary operands (W1p rows + bias row)
        a1w = state.tile([3, 20, 2, 128], MDT, tag="a1w")
        nc.gpsimd.dma_start(out=a1w[0:2], in_=ins["a1top"])
        nc.gpsimd.dma_start(out=a1w[2:3].rearrange("a b c d -> a (b c d)"),
                            in_=brows)

        # ================= CNF: euler + gen =================
        if "cnf" not in PHASES:
            return
        # Software-pipelined emission: streams processed in pairs (matching
        # the 2-slot PSUM rings); each pair's tail ops (ups/vW/scr/update)
        # are emitted during the NEXT pair's head so every engine queue has
        # ready work at its head (engines dispatch strictly in order).
        # Squares are split mb0->Pool / mb1->DVE to balance those engines.
        with tc.tile_pool(name="psA", bufs=2, space="PSUM") as psA, \
                tc.tile_pool(name="psB", bufs=2, space="PSUM") as psB:
            pairs = [(0, (0, 1)), (1, (0, 1)), (0, (2, 3)), (1, (2, 3))]
            for i in range(STEPS):
                h1sqs, upss, h2sqs, vpss = {}, {}, {}, {}
                a2s, h2s = {}, {}

                def emit_ups(nts):
                    for nt in nts:
                        h1sqv = h1sqs[nt].rearrange("p (k n) -> p k n", k=2)
                        ups = psA.tile([128, 2 * NT], F32, tag="pA", bufs=2)
                        for mb in range(2):
                            mbs = slice(mb * 128, (mb + 1) * 128)
                            om = ups[:, mb * NT:(mb + 1) * NT]
                            nc.tensor.matmul(om, _mm(c["nWu8"][:, :, mbs]),
                                             _mm(h1sqv), start=True,
                                             stop=False, perf_mode=DR)
                            nc.tensor.matmul(om, _mm(c["rub"][:, mbs]),
                                             _mm(c["ones2b"][0:1, 0:NT]),
                                             start=False, stop=True)
                        upss[nt] = ups

                def emit_tail(half, nts):
                    for nt in nts:
                        vw = c["vW8"] if half == 0 else c["nvW8"]
                        h2v = h2s[(half, nt)].rearrange("p (k n) -> p k n",
                                                        k=2)
                        vps = psB.tile([16, NT], F32, tag="pB", bufs=2)
                        nc.tensor.matmul(vps, _mm(vw), _mm(h2v),
                                         start=True, stop=True, perf_mode=DR)
                        vpss[(half, nt)] = vps
                    if half == 0:
                        for nt in nts:
                            # scr = (h2sq - 1) * ups = -s2 * t
                            # divslots col = sum(scr); host negates.
                            slot = i * NNT + nt
                            scr = work.tile([128, 2 * NT], BF16, tag="scr",
                                            bufs=3)
                            nc.vector.scalar_tensor_tensor(
                                out=scr, in0=h2sqs[nt], scalar=1.0,
                                in1=upss[nt], op0=ALU.subtract, op1=ALU.mult,
                                accum_out=divslots[:, slot:slot + 1])
                    for nt in nts:
                        stl = ybuf if half == 0 else sbuf_
                        st = stl[i % 2][nt]
                        stn = stl[(i + 1) % 2][nt]
                        nc.vector.tensor_tensor(stn[0:2, :], st[0:2, :],
                                                vpss[(half, nt)][0:2, :],
                                                ALU.add)

                prev = None
                for half, nts in pairs:
                    stl = ybuf if half == 0 else sbuf_
                    evi = i if half == 0 else STEPS + i
                    a1s = {}
                    for nt in nts:
                        st = stl[i % 2][nt]
                        a1 = psA.tile([128, 2 * NT], F32, tag="pA", bufs=2)
                        for mb in range(2):
                            nc.tensor.matmul(a1[:, mb * NT:(mb + 1) * NT],
                                             _mm(a1w[:, evi, mb, :]),
                                             _mm(st[0:3, :]),
                                             start=True, stop=True)
                        a1s[nt] = a1
                    if prev is not None and prev[0] == 0:
                        emit_ups(prev[1])
                    h1s = {}
                    for nt in nts:
                        h1 = work.tile([128, 2 * NT], F8, tag="h1", bufs=4)
                        nc.scalar.activation(h1, a1s[nt], AF.Tanh)
                        h1s[nt] = h1
                        if half == 0:
                            h1sq = work.tile([128, 2 * NT], F8, tag="h1sq",
                                             bufs=3)
                            nc.gpsimd.tensor_tensor(h1sq[:, 0:NT],
                                                    h1[:, 0:NT],
                                                    h1[:, 0:NT], ALU.mult)
                            nc.vector.tensor_tensor(h1sq[:, NT:], h1[:, NT:],
                                                    h1[:, NT:], ALU.mult)
                            h1sqs[nt] = h1sq
                    for nt in nts:
                        h1v = h1s[nt].rearrange("p (k n) -> p k n", k=2)
                        a2 = psB.tile([128, 2 * NT], F32, tag="pB", bufs=2)
                        for mb in range(2):
                            mbs = slice(mb * 128, (mb + 1) * 128)
                            nc.tensor.matmul(a2[:, mb * NT:(mb + 1) * NT],
                                             _mm(c["W28"][:, :, mbs]),
                                             _mm(h1v), start=True, stop=True,
                                             perf_mode=DR)
                        a2s[(half, nt)] = a2
                    if prev is not None:
                        emit_tail(*prev)
                    for nt in nts:
                        h2 = work.tile([128, 2 * NT], F8, tag="h2", bufs=4)
                        a2 = a2s[(half, nt)]
                        for mb in range(2):
                            ms = slice(mb * NT, (mb + 1) * NT)
                            nc.scalar.activation(h2[:, ms], a2[:, ms],
                                                 AF.Tanh,
                                                 bias=c["b2c"][:, mb:mb + 1])
                        h2s[(half, nt)] = h2
                        if half == 0:
                            h2sq = work.tile([128, 2 * NT], BF16, tag="h2sq",
                                             bufs=3)
                            nc.gpsimd.tensor_tensor(h2sq[:, 0:NT],
                                                    h2[:, 0:NT],
                                                    h2[:, 0:NT], ALU.mult)
                            nc.vector.tensor_tensor(h2sq[:, NT:], h2[:, NT:],
                                                    h2[:, NT:], ALU.mult)
                            h2sqs[nt] = h2sq
                    prev = (half, nts)
                emit_ups_last = None  # gen pairs have no ups
                emit_tail(*prev)

        nc.vector.tensor_reduce(divacc, divslots, axis=AX.X, op=ALU.add)
        nc.sync.dma_start(out=outs["o_div"], in_=divacc)

        # final y stats: y_true = y + b3 (per n-tile)
        sy2slots = small.tile([2, NNT], F32, tag="sy2slots")
        for nt in range(NNT):
            ytrue = work.tile([2, NT], F32, tag="yt", name="ytrue")
            nc.vector.tensor_scalar(ytrue, ybuf[STEPS % 2][nt][0:2, :],
                                    c["b3c"], None, ALU.add)
            sy2scr = work.tile([2, NT], F32, tag="scr2", name="sy2scr")
            nc.scalar.activation(sy2scr, ytrue, AF.Square,
                                 accum_out=sy2slots[:, nt:nt + 1])
        sy2 = small.tile([2, 1], F32, tag="sy2")
        nc.vector.tensor_reduce(sy2, sy2slots, axis=AX.X, op=ALU.add)
        nc.sync.dma_start(out=outs["o_sy2"], in_=sy2)

        # ================= chamfer =================
        if "cham" not in PHASES:
            return
        # D[p,m] = |r_p - x_m|^2 in bf16-compensated form: both clouds are
        # rounded to bf16 (smooth, unbiased shift), squared norms split
        # hi/lo so all PE products are exact in fp32 accumulation.
        # Side A (per-r min): lhsT raugA [rh_x, rh_y, sqr_hi, sqr_lo, 1, 1]
        #   x rhs xaug [-2xh_x, -2xh_y, 1, 1, sqx_hi, sqx_lo].
        # Side B (per-x min): lhsT xplain [1, 1, sqx_hi, sqx_lo, xh_x, xh_y]
        #   x rhs raugB [sqr_hi, sqr_lo, 1, 1, -2rh_x, -2rh_y].
        raugA = state.tile([6, N], BF16, tag="raugA")
        raugB = state.tile([6, N], BF16, tag="raugB")
        # engine ops need partition base in {0,32,64,96}: build rows in
        # base-0 tiles, assemble raugA/raugB via SBUF->SBUF DMA.
        rh = work.tile([2, N], BF16, tag="rh", bufs=1, name="rh")
        for nt in range(NNT):
            sl = slice(nt * NT, (nt + 1) * NT)
            nc.vector.tensor_scalar(rh[:, sl],
                                    sbuf_[STEPS % 2][nt][0:2, :],
                                    c["nb3c"], None, ALU.add)
        sqpair = work.tile([2, N], F32, tag="sqp", bufs=1, name="sqpair")
        nc.vector.scalar_tensor_tensor(out=sqpair, in0=rh, scalar=1.0,
                                       in1=rh, op0=ALU.mult, op1=ALU.mult)
        sqr = work.tile([1, N], F32, tag="sqr", bufs=1, name="sqr")
        nc.gpsimd.tensor_reduce(sqr, sqpair, axis=AX.C, op=ALU.add)
        shi = work.tile([1, N], BF16, tag="shi", bufs=1, name="shi")
        nc.vector.tensor_copy(shi, sqr)                 # sqr_hi (bf16 round)
        slo = work.tile([1, N], BF16, tag="slo", bufs=1, name="slo")
        nc.vector.tensor_tensor(slo, sqr, shi, ALU.subtract)
        nh2 = work.tile([2, N], BF16, tag="nh2", bufs=1, name="nh2")
        nc.vector.tensor_scalar(nh2, rh, -2.0, None, ALU.mult)
        nc.sync.dma_start(out=raugA[0:2], in_=rh)
        nc.sync.dma_start(out=raugA[2:3], in_=shi)
        nc.sync.dma_start(out=raugA[3:4], in_=slo)
        nc.sync.dma_start(out=raugA[4:6], in_=ins["ones2b"])
        nc.sync.dma_start(out=raugB[0:1], in_=shi)
        nc.sync.dma_start(out=raugB[1:2], in_=slo)
        nc.sync.dma_start(out=raugB[2:4], in_=ins["ones2b"])
        nc.sync.dma_start(out=raugB[4:6], in_=nh2)

        chAmin = small.tile([128, 16], F32, tag="chAmin")
        chBmin = small.tile([128, 16], F32, tag="chBmin")
        with tc.tile_pool(name="psD", bufs=2, space="PSUM") as psD:
            for blk in range(16):
                bsl = slice(blk * 128, (blk + 1) * 128)
                Dp = psD.tile([128, N], F32, tag="D")
                for mt in range(NNT):
                    msl = slice(mt * NT, (mt + 1) * NT)
                    nc.tensor.matmul(Dp[:, msl], raugA[:, bsl],
                                     c["xaug"][:, msl], start=True, stop=True)
                nc.vector.tensor_reduce(chAmin[:, blk:blk + 1], Dp,
                                        axis=AX.X, op=ALU.min)
                Dq = psD.tile([128, N], F32, tag="D")
                for mt in range(NNT):
                    msl = slice(mt * NT, (mt + 1) * NT)
                    nc.tensor.matmul(Dq[:, msl], c["xplain"][:, bsl],
                                     raugB[:, msl], start=True, stop=True)
                nc.vector.tensor_reduce(chBmin[:, blk:blk + 1], Dq,
                                        axis=AX.X, op=ALU.min)
        # clamp, sqrt, per-partition sums over the 16 blocks
        chs = small.tile([128, 16], F32, tag="chs")
        red = small.tile([128, 1], F32, tag="red")
        nc.vector.tensor_scalar_max(chAmin, chAmin, 0.0)
        nc.scalar.activation(chs, chAmin, AF.Sqrt)
        nc.vector.tensor_reduce(red, chs, axis=AX.X, op=ALU.add)
        nc.sync.dma_start(out=outs["o_chA"], in_=red)
        chs2 = small.tile([128, 16], F32, tag="chs2")
        red2 = small.tile([128, 1], F32, tag="red2")
        nc.vector.tensor_scalar_max(chBmin, chBmin, 0.0)
        nc.scalar.activation(chs2, chBmin, AF.Sqrt)
        nc.vector.tensor_reduce(red2, chs2, axis=AX.X, op=ALU.add)
        nc.sync.dma_start(out=outs["o_chB"], in_=red2)


_NC_CACHE = {}


def _get_nc():
    if "nc" not in _NC_CACHE:
        _NC_CACHE["nc"] = build_nc()
    return _NC_CACHE["nc"]


def build_in_maps(inputs, pre):
    ones_row = np.ones((1, N), np.float32)
    in_maps = []
    for b in range(B):
        m = dict(pre)
        m["xT3"] = np.ascontiguousarray(
            np.concatenate([inputs["x"][b].T, ones_row], 0), np.float32)
        m["nT3"] = np.ascontiguousarray(
            np.concatenate([inputs["noise"][b].T, ones_row], 0), np.float32)
        m["epsc"] = np.ascontiguousarray(inputs["eps"][b][:, None], np.float32)
        m["xaug"], m["xplain"] = host_cham(inputs["x"][b])
        in_maps.append(m)
    return in_maps


def kernel(**inputs):
    inputs = {k: np.asarray(v, dtype=np.float32) if np.asarray(v).dtype != np.int32
              else np.asarray(v) for k, v in inputs.items()}
    pre = host_precompute(inputs)
    nc = _get_nc()
    in_maps = build_in_maps(inputs, pre)
    res = run_bass_kernel_spmd(nc, in_maps, core_ids=list(range(B)))
    return combine(res.results, pre)


def combine(results, pre):
    S_logpy = 0.0
    S_logdet = 0.0
    prior = 0.0
    entropy = 0.0
    chamA = 0.0
    chamB = 0.0
    for r in results:
        S_logpy += -0.5 * float(r["o_sy2"].sum()) - N * LOG2PI
        # device computes divslot = sum((h2sq-1)*(ru - Wu@h1sq)) = -sum(s2*t)
        S_logdet += DT * (-float(r["o_div"].sum()))
        mu = r["o_mu"].astype(np.float64)
        lv = r["o_lv"].astype(np.float64)
        prior += 0.5 * float((mu ** 2 + np.exp(lv) - lv - 1.0).sum())
        entropy += -0.5 * float((lv + 1.0 + LOG2PI).sum())
        chamA += float(r["o_chA"].sum())
        chamB += float(r["o_chB"].sum())
    recon = -(S_logpy + S_logdet) / (B * N)
    prior /= B
    entropy /= B
    cham = chamA / (B * N) + chamB / (B * N)
    vol = max(0.0, S_logdet / (B * N) - 10.0)
    return np.float32(LAM_R * recon + LAM_P * prior + LAM_E * entropy
                      + LAM_C * cham + LAM_V * vol)
